# revision 81
# baseline (speedup 1.0000x reference)
"""Multi-head causal attention (B=4, S=2048, D=1024, 16 heads) on 8 TRN2 cores.

Sharding: core c -> (batch b = c//2, head-group g = c%2). Each core computes
8 heads of one batch element end-to-end (QKV proj, causal softmax attention,
out-proj rows for its head slice). Host sums the two head-group partials per
batch and adds the output bias.

Per-core pipeline (all matmuls contraction-on-partitions, bf16 in / f32 psum):
  QT/KT[dtile] = (x @ w)^T   [128p = 2 heads x 64, S]
  Vones[kb]    = [V | 1]     [128p = k, h, 65]
  attention per (512-wide q-chunk, head-pair); the pair's score matmuls are
  packed into PE row groups via tile_position; two k-blocks share one
  [128,1024] score psum so each exp covers ~1024 cols; PV matmuls trail two
  iterations behind the exp:
    ST[k,q] = KT.T @ QT; PT = exp(ST/8) bf16; tri-mask on diagonal 128 cols
    ctx[q 128, 65] += PT_slice.T @ [V_h | 1]   (col 64 = softmax denominator)
      -- flipped PV orientation: out free = 65 instead of 512, so the PE pays
         65*nkb cycles per q-tile instead of 512*nkb per q-chunk (2x less).
    norm: DVE reciprocal of denom col + tensor_mul with free-dim broadcast
    transpose [128 q, 128 dpair] -> cxtq tiles via DMA xbar transpose
      (runs on the DMA engines; no PE/DVE cost)
  out[seq128, 512] = cxtq.T @ ow, streamed to DRAM per (sq, oc) group.
  V-proj k-blocks 4..15 and the out-proj groups are deferred into the later
  (exp-bound) q-chunk windows so the PE always has filler work while the
  ACT engine grinds through the exps.
"""

import numpy as np
import ml_dtypes

B, S, D = 4, 2048, 1024
H_TOT = 16
HD = 64
NCORES = 8
GH = 8          # heads per core
GD = GH * HD    # 512: dout slice per core
NKB = S // 128  # 16 k-blocks
NQC = S // 512  # 4 q-chunks
BF16 = ml_dtypes.bfloat16

PACK_HEADS = True   # pack 2 heads' score matmuls into PE row groups

_cache = {}


def _build_body(tc, nc, mybir, xT, wq, wk, wv, ow, outp):
    from concourse.masks import make_upper_triangular
    import contextlib

    dt = mybir.dt
    F = mybir.ActivationFunctionType

    pools = contextlib.ExitStack()
    tc_pool = lambda **kw: pools.enter_context(tc.tile_pool(**kw))

    singles = tc_pool(name="singles", bufs=1)
    pt_pool = tc_pool(name="pt", bufs=6)
    pt3_pool = tc_pool(name="pt3", bufs=16)   # prefetched pairs (3,0)/(3,1)
    pt32_pool = tc_pool(name="pt32", bufs=8)  # (3,2) half-prefetch
    rec_pool = tc_pool(name="rec", bufs=4)
    nq_pool = tc_pool(name="nq", bufs=2)
    ost_pool = tc_pool(name="ost", bufs=4)
    prt_pool = tc_pool(name="prt", bufs=8)    # out-proj dvt{0,1} partials
    psum_st = tc_pool(name="psum_st", bufs=2, space="PSUM")   # 2 x 2 banks
    psum_cx = tc_pool(name="psum_cx", bufs=4, space="PSUM")   # 4 x 1 bank

    # ---- persistent SBUF tensors ----
    xT_sb = [singles.tile([128, S], dt.bfloat16, name=f"xt{t}")
             for t in range(8)]
    wq_sb = [singles.tile([128, GD], dt.bfloat16, name=f"wq{t}")
             for t in range(8)]
    wk_sb = [singles.tile([128, GD], dt.bfloat16, name=f"wk{t}")
             for t in range(8)]
    wv_sb = [singles.tile([128, GD], dt.bfloat16, name=f"wv{t}")
             for t in range(8)]
    ow_sb = [singles.tile([128, D], dt.bfloat16, name=f"ow{t}")
             for t in range(4)]
    qt_sb = [singles.tile([128, S], dt.bfloat16, name=f"qt{t}")
             for t in range(4)]                              # 2 heads / dtile
    kt_sb = [singles.tile([128, S], dt.bfloat16, name=f"kt{t}")
             for t in range(4)]
    vo_sb = [singles.tile([128, GH, 65], dt.bfloat16, name=f"vo{t}")
             for t in range(NKB)]                            # [V_h | ones]
    # normalized ctx^T, one [128 dpair, 128 q] tile per (head-pair, seq-tile):
    # whole-tile DMA-transpose destinations must be contiguous SBUF.
    cxtq = [[singles.tile([128, 128], dt.bfloat16, name=f"cq{hp}_{sq}")
             for sq in range(16)] for hp in range(4)]
    tri = singles.tile([128, 128], dt.bfloat16)              # keep k<=q
    ident = singles.tile([128, 128], dt.bfloat16)

    from concourse.masks import make_identity
    make_upper_triangular(nc, tri, val=1.0, diag=True)
    make_identity(nc, ident)
    for t in range(NKB):
        nc.vector.memset(vo_sb[t][:, :, 64:65], 1.0)

    # ---- input DMAs ----
    # xT split in column halves so projections can start on partial tiles.
    xT_r = xT.ap().rearrange("(t p) (h s) -> p t h s", p=128, h=2)
    wq_r = wq.ap().rearrange("(t p) n -> p t n", p=128)
    wk_r = wk.ap().rearrange("(t p) n -> p t n", p=128)
    wv_r = wv.ap().rearrange("(t p) n -> p t n", p=128)
    ow_r = ow.ap().rearrange("(t p) n -> p t n", p=128)
    DIN_ORDER = list(range(8))
    for t in range(8):
        nc.sync.dma_start(out=wq_sb[t], in_=wq_r[:, t, :])
        nc.sync.dma_start(out=xT_sb[t][:, 0:1024], in_=xT_r[:, t, 0, :])
        nc.sync.dma_start(out=wk_sb[t], in_=wk_r[:, t, :])
    for t in range(8):
        nc.sync.dma_start(out=xT_sb[t][:, 1024:2048], in_=xT_r[:, t, 1, :])
    for t in range(8):
        nc.sync.dma_start(out=wv_sb[t], in_=wv_r[:, t, :])
    # ow is needed late (first out-proj group sits two q-chunk windows in);
    # issue from the Pool SWDGE path, gated behind the last xT tile so its
    # transfers never steal DMA-engine slots from the startup-critical loads.
    gate = singles.tile([128, 1], dt.bfloat16)
    nc.gpsimd.tensor_copy(out=gate, in_=xT_sb[7][:, 0:1])
    for t in range(4):
        nc.gpsimd.dma_start(out=ow_sb[t], in_=ow_r[:, t, :])

    def emit_proj_pass(w_sb, t_sb, dts, ch):
        """One projection (Q or K) for dtiles `dts` over xT column half
        `ch`, din-outer so the PE consumes ~1.7us per din — matched to the
        input DMA rate.  Uses only the 1-bank cx pool, so the stp pool stays
        free for score/exp prefetches at any point."""
        pss = [psum_cx.tile([128, 512], dt.float32, name="cxp")
               for _ in range(4)]
        for i, din in enumerate(DIN_ORDER):
            for j, dtile in enumerate(dts):
                dsl = slice(dtile * 128, (dtile + 1) * 128)
                for c in range(2):
                    rhs = xT_sb[din][:, ch * 1024 + c * 512:
                                     ch * 1024 + (c + 1) * 512]
                    nc.tensor.matmul(pss[2 * j + c], lhsT=w_sb[din][:, dsl],
                                     rhs=rhs, start=(i == 0), stop=(i == 7))
        for j, dtile in enumerate(dts):
            for c in range(2):
                col = ch * 1024 + c * 512
                nc.vector.tensor_copy(
                    out=t_sb[dtile][:, col:col + 512], in_=pss[2 * j + c])

    def emit_proj_pass_qk(dts, ch):
        """Merged Q+K pass (cx pool + stp pool): 8 matmuls per din, 2x the
        input DMA rate — used for the first dtile pair while xT streams in
        and no scores need the stp pool yet."""
        qs = [psum_cx.tile([128, 512], dt.float32, name="cxp")
              for _ in range(4)]
        kst = [psum_st.tile([128, 1024], dt.float32, name="stp")
               for _ in range(2)]
        ks = [kst[0][:, 0:512], kst[0][:, 512:1024],
              kst[1][:, 0:512], kst[1][:, 512:1024]]
        for i, din in enumerate(DIN_ORDER):
            for j, dtile in enumerate(dts):
                dsl = slice(dtile * 128, (dtile + 1) * 128)
                for c in range(2):
                    rhs = xT_sb[din][:, ch * 1024 + c * 512:
                                     ch * 1024 + (c + 1) * 512]
                    nc.tensor.matmul(qs[2 * j + c], lhsT=wq_sb[din][:, dsl],
                                     rhs=rhs, start=(i == 0), stop=(i == 7))
                    nc.tensor.matmul(ks[2 * j + c], lhsT=wk_sb[din][:, dsl],
                                     rhs=rhs, start=(i == 0), stop=(i == 7))
        for j, dtile in enumerate(dts):
            for c in range(2):
                col = ch * 1024 + c * 512
                nc.vector.tensor_copy(
                    out=qt_sb[dtile][:, col:col + 512], in_=qs[2 * j + c])
                nc.vector.tensor_copy(
                    out=kt_sb[dtile][:, col:col + 512], in_=ks[2 * j + c])

    v_ps = {}

    def emit_v_half(st, half):
        """One 4-din half of a V-projection k-block (a ~0.85us PE unit)."""
        if half == 0:
            v_ps[st] = psum_cx.tile([128, 512], dt.float32, name="cxp")
        ps = v_ps[st]
        for din in range(4 * half, 4 * half + 4):
            nc.tensor.matmul(
                ps,
                lhsT=xT_sb[din][:, st * 128:(st + 1) * 128],
                rhs=wv_sb[din],
                start=(din == 0),
                stop=(din == 7),
            )
        if half == 1:
            nc.vector.tensor_copy(
                out=vo_sb[st][:, :, 0:64],
                in_=ps.rearrange("p (h d) -> p h d", h=GH),
            )
            del v_ps[st]

    def emit_v(st):
        emit_v_half(st, 0)
        emit_v_half(st, 1)

    class PairState:
        """One (q-chunk, head-pair) attention unit, split so the scheduler
        can interleave its score/exp iterations and PV drain with other PE
        work.  Scores pack 2 heads into PE row groups; two k-blocks share a
        [128,1024] ST psum so each exp covers up to 1024 columns.  PV
        orientation: ctx[q 128, 65] += PT_slice.T @ [V|1] per (head, q-tile)
        so each PV matmul costs only 65 PE rows (col 64 = denominators)."""

        def __init__(self, qc, hp, pool=None):
            self.qc, self.hp = qc, hp
            self.pool = pool or pt_pool
            self.nkb = 4 * qc + 4
            self.q0 = 512 * qc
            self.pend = []
            self.kb_iter = iter(range(0, self.nkb, 2))
            self.ctxs = None

        def step_scores(self):
            kb0 = next(self.kb_iter, None)
            if kb0 is None:
                return False
            qc, hp, q0 = self.qc, self.hp, self.q0
            kbs = [kb for kb in (kb0, kb0 + 1) if kb < self.nkb]
            ns = [512 - max(0, kb * 128 - q0) for kb in kbs]
            offs = [0] + [ns[0]] * (len(kbs) - 1)
            pts = []
            for half in range(2):
                p0 = half * 64
                stp = psum_st.tile([128, 1024], dt.float32, name="stp")
                for kb, off, n in zip(kbs, offs, ns):
                    nc.tensor.matmul(
                        stp[:, off:off + n],
                        lhsT=kt_sb[hp][p0:p0 + 64, kb * 128:(kb + 1) * 128],
                        rhs=qt_sb[hp][p0:p0 + 64, q0 + 512 - n:q0 + 512],
                        start=True,
                        stop=True,
                        tile_position=(p0, 0) if PACK_HEADS else None,
                    )
                ntot = offs[-1] + ns[-1]
                pt = self.pool.tile([128, 1024], dt.bfloat16,
                                    name=self.pool.name)
                nc.scalar.activation(
                    out=pt[:, :ntot], in_=stp[:, :ntot], func=F.Exp,
                    scale=0.125)
                for kb, off in zip(kbs, offs):
                    if kb >= 4 * qc:  # diagonal: mask first 128 cols
                        nc.vector.tensor_mul(
                            pt[:, off:off + 128], pt[:, off:off + 128], tri)
                pts.append(pt)
            self.pend.append((kbs, offs, ns, pts))
            return True

        def pv_one(self):
            if self.ctxs is None:
                self.ctxs = [psum_cx.tile([128, 4, 65], dt.float32,
                                          name="cxp") for _ in range(2)]
            kbs, offs, ns, pts = self.pend.pop(0)
            qc, hp = self.qc, self.hp
            # One psum accumulation group per ctx BANK: only the pair's very
            # first PV carries start (the bank-wide pending-zero mark zeroes
            # every qt slice's first write), only the final PV carries stop.
            for half in range(2):
                for (kb, off, n) in zip(kbs, offs, ns):
                    for qt in range(4):
                        qt_g = 4 * qc + qt
                        if kb > qt_g:
                            continue
                        col = off + qt * 128 - (512 - n)
                        nc.tensor.matmul(
                            self.ctxs[half][:, qt, :],
                            lhsT=pts[half][:, col:col + 128],
                            rhs=vo_sb[kb][:, 2 * hp + half, :],
                            start=(kb == 0 and qt == 0),
                            stop=(qt == 3 and kb == 4 * qc + 3),
                        )

        def finish(self, pe_transpose=False):
            while self.pend:
                self.pv_one()
            qc, hp = self.qc, self.hp
            normq = nq_pool.tile([128, 4, 2, 64], dt.bfloat16, name="normq")
            for half in range(2):
                recip = rec_pool.tile([128, 4], dt.float32, name="recip")
                nc.vector.reciprocal(out=recip, in_=self.ctxs[half][:, :, 64])
                nc.vector.tensor_mul(
                    normq[:, :, half, :],
                    self.ctxs[half][:, :, 0:64],
                    recip[:, :, None].broadcast_to([128, 4, 64]),
                )
            if pe_transpose:
                # last pair: the ~2us DMA-transpose latency would sit on the
                # critical path, so transpose on the PE instead; copies split
                # across DVE and Pool to shorten the drain chain.
                trp = psum_cx.tile([128, 512], dt.bfloat16, name="cxp")
                for qt in (3, 2, 1, 0):
                    nc.tensor.transpose(
                        trp[:, qt * 128:(qt + 1) * 128],
                        normq[:, qt, :, :], ident)
                    # gpsimd cannot read PSUM; split DVE/ACT instead
                    if qt % 2 == 0:
                        nc.vector.tensor_copy(
                            out=cxtq[hp][4 * qc + qt],
                            in_=trp[:, qt * 128:(qt + 1) * 128])
                    else:
                        nc.scalar.activation(
                            out=cxtq[hp][4 * qc + qt],
                            in_=trp[:, qt * 128:(qt + 1) * 128], func=F.Copy)
            else:
                for qt in range(4):
                    nc.sync.dma_start_transpose(
                        out=cxtq[hp][4 * qc + qt],
                        in_=normq[:, qt, :, :],
                    )

    def emit_attn_pair(qc, hp):
        ps = PairState(qc, hp)
        while ps.step_scores():
            if len(ps.pend) > 2:
                ps.pv_one()
        ps.finish()

    op_ost = {}
    op_ps = {}

    def emit_op_quarter(sq, oc, dq, pool=None):
        """Two-dvt quarter of an out-proj half-group (a ~0.43us PE unit);
        the two oc halves share one ost tile and a merged output DMA."""
        if dq == 0:
            op_ps[(sq, oc)] = (pool or psum_cx).tile(
                [128, 512], dt.float32,
                name="stp" if pool is psum_st else "cxp")
        ps = op_ps[(sq, oc)]
        for dvt in (2 * dq, 2 * dq + 1):
            nc.tensor.matmul(
                ps,
                lhsT=cxtq[dvt][sq],
                rhs=ow_sb[dvt][:, oc * 512:(oc + 1) * 512],
                start=(dvt == 0),
                stop=(dvt == 3),
            )
        if dq == 1:
            del op_ps[(sq, oc)]
            if sq not in op_ost:
                op_ost[sq] = ost_pool.tile([128, 1024], dt.bfloat16,
                                           name="ost")
            ost = op_ost[sq]
            nc.vector.tensor_copy(out=ost[:, oc * 512:(oc + 1) * 512],
                                  in_=ps)
            if oc == 1:
                nc.sync.dma_start(
                    out=outp.ap()[sq * 128:(sq + 1) * 128, :],
                    in_=ost,
                )
                del op_ost[sq]

    def emit_op(sq, oc, pool):
        emit_op_quarter(sq, oc, 0, pool)
        emit_op_quarter(sq, oc, 1, pool)

    def emit_op_partial(sq, oc):
        """dvt 0..2 partial of an out-proj group, parked in SBUF as bf16;
        lets rows 12..15 run ahead so only the dvt=3 slice trails (3,3)."""
        ps = psum_cx.tile([128, 512], dt.float32, name="cxp")
        for dvt in range(3):
            nc.tensor.matmul(
                ps,
                lhsT=cxtq[dvt][sq],
                rhs=ow_sb[dvt][:, oc * 512:(oc + 1) * 512],
                start=(dvt == 0),
                stop=(dvt == 2),
            )
        prt = prt_pool.tile([128, 512], dt.bfloat16, name="prt")
        nc.vector.tensor_copy(out=prt, in_=ps)
        return prt

    def emit_op_rest(sq, oc, prt, pool, copy_eng):
        """Re-inject the bf16 partial through the PE (identity matmul) and
        accumulate the dvt=3 slice on top, so the tail needs only a copy."""
        ps = pool.tile([128, 512], dt.float32,
                       name="stp" if pool is psum_st else "cxp")
        nc.tensor.matmul(ps, lhsT=ident, rhs=prt, start=True, stop=False)
        nc.tensor.matmul(
            ps,
            lhsT=cxtq[3][sq],
            rhs=ow_sb[3][:, oc * 512:(oc + 1) * 512],
            start=False,
            stop=True,
        )
        if sq not in op_ost:
            op_ost[sq] = ost_pool.tile([128, 1024], dt.bfloat16, name="ost")
        ost = op_ost[sq]
        if copy_eng is nc.scalar:
            nc.scalar.activation(out=ost[:, oc * 512:(oc + 1) * 512],
                                 in_=ps, func=F.Copy)
        else:
            copy_eng.tensor_copy(out=ost[:, oc * 512:(oc + 1) * 512], in_=ps)
        if oc == 1:
            nc.sync.dma_start(
                out=outp.ap()[sq * 128:(sq + 1) * 128, :],
                in_=ost,
            )
            del op_ost[sq]

    def weave_ps(ps, fillers, pe_transpose=False, keep=2):
        """Interleave filler callables between score iterations so the PE
        keeps pace while the ACT engine works through the exps.  Pop rate
        adapts so the fillers spread evenly, holding two back to cover the
        final exps' latency before the PV drain."""
        iters_left = len(list(range(0, ps.nkb, 2))) - len(ps.pend)
        i = 0
        while ps.step_scores():
            i += 1
            iters_left -= 1
            if len(ps.pend) > 2:
                ps.pv_one()
            if i % 2 == 0 and iters_left > 0:
                npop = max(0, (len(fillers) - keep) * 2 // (iters_left + 1))
                for _ in range(max(2, npop)):
                    if len(fillers) > keep:
                        fillers.pop(0)()
        while len(ps.pend) > 1:
            ps.pv_one()
        for f in fillers:
            f()
        ps.finish(pe_transpose=pe_transpose)

    def weave_pair(qc, hp, fillers, pe_transpose=False, keep=2):
        weave_ps(PairState(qc, hp), fillers, pe_transpose=pe_transpose,
                 keep=keep)

    def op_fillers(*sqocs):
        return [(lambda sq=sq, oc=oc, dq=dq: emit_op_quarter(sq, oc, dq))
                for sq, oc in sqocs for dq in range(2)]

    def v_fillers(*sts):
        return [(lambda st=st, h=h: emit_v_half(st, h))
                for st in sts for h in range(2)]

    # ---- emission schedule ----
    # Pair (3,0)'s scores/exps are prefetched into the (ACT-idle) projection
    # window; its PT tiles persist in SBUF (own pool) and its PVs run at the
    # end of w1.  V-proj blocks and out-proj groups are spread as PE filler
    # through the exp-bound later windows.
    p30 = PairState(3, 0, pool=pt3_pool)
    p32 = PairState(3, 2, pool=pt32_pool)

    # w0: QK projections (DMA-paced, cx-pool-only passes) + qc=0 attention +
    # all of (3,0)'s and half of (3,2)'s scores in the otherwise-idle ACT
    # stretch, with V-proj halves as PE filler.
    emit_proj_pass_qk((0, 1), 0)
    emit_proj_pass_qk((0, 1), 1)
    p00 = PairState(0, 0)
    p00.step_scores()
    p00.step_scores()
    emit_v(0)
    emit_v(1)
    p01 = PairState(0, 1)
    p01.step_scores()
    p01.step_scores()
    emit_v(2)
    emit_v(3)
    p00.finish()
    p30.step_scores()
    p30.step_scores()
    p01.finish()
    emit_proj_pass(wq_sb, qt_sb, (2, 3), 0)
    p30.step_scores()
    p30.step_scores()
    emit_proj_pass(wq_sb, qt_sb, (2, 3), 1)
    p30.step_scores()
    p30.step_scores()
    emit_proj_pass(wk_sb, kt_sb, (2, 3), 0)
    p30.step_scores()
    p30.step_scores()
    emit_proj_pass(wk_sb, kt_sb, (2, 3), 1)
    p02 = PairState(0, 2)
    p02.step_scores()
    emit_v_half(4, 0)
    emit_v_half(4, 1)
    p02.step_scores()
    emit_v_half(5, 0)
    emit_v_half(5, 1)
    p32.step_scores()
    emit_v_half(6, 0)
    emit_v_half(6, 1)
    p02.finish()
    p03 = PairState(0, 3)
    p03.step_scores()
    emit_v_half(7, 0)
    emit_v_half(7, 1)
    p03.step_scores()
    p32.step_scores()
    emit_v_half(8, 0)
    emit_v_half(8, 1)
    p03.finish()
    # w1 (qc=1): V 9..15 and (3,2)'s remaining prefetch woven in, then the
    # (3,0) PV drain paced against a prefetch of half of (3,1)'s scores
    # (which reuses the pt3 slots the drain frees).
    p10 = PairState(1, 0)
    p10.step_scores()
    p10.step_scores()
    emit_v_half(9, 0)
    emit_v_half(9, 1)
    p10.step_scores()
    p10.pv_one()
    p32.step_scores()
    emit_v_half(10, 0)
    emit_v_half(10, 1)
    p10.step_scores()
    p10.pv_one()
    p32.step_scores()
    emit_v_half(11, 0)
    emit_v_half(11, 1)
    p10.finish()
    weave_pair(1, 1, v_fillers(12, 13))
    weave_pair(1, 2, v_fillers(14, 15))
    weave_pair(1, 3, op_fillers((0, 0)))
    p31 = PairState(3, 1, pool=pt3_pool)
    opf0 = op_fillers((0, 1), (1, 0))
    for _ in range(4):
        p31.step_scores()
        for _ in range(2):
            if p30.pend:
                p30.pv_one()
        if opf0:
            opf0.pop(0)()
    p30.finish()
    for f in opf0:
        f()
    # w2 (qc=2): out-proj rows 1..7 woven in as filler; during (2,3) the
    # dvt{0..2} partials of rows 8..9 start early (they need only pairs
    # (2,0)..(2,2)), with their dvt=3 rests as w3 filler.
    parts = {}
    def part_filler(sq, oc):
        def f():
            parts[(sq, oc)] = emit_op_partial(sq, oc)
        return f
    def rest_filler(sq, oc):
        def f():
            emit_op_rest(sq, oc, parts[(sq, oc)], psum_cx, nc.vector)
        return f
    weave_pair(2, 0, op_fillers((1, 1), (2, 0), (2, 1), (3, 0)))
    weave_pair(2, 1, op_fillers((3, 1), (4, 0), (4, 1)))
    weave_pair(2, 2, op_fillers((5, 0), (5, 1), (6, 0), (6, 1)))
    weave_pair(2, 3, op_fillers((7, 0), (7, 1))
               + [part_filler(8, 0), part_filler(8, 1),
                  part_filler(9, 0), part_filler(9, 1)], keep=3)
    # w3 (qc=3, pairs 1..3): partials/rests of rows 8..11 + dvt{0..2}
    # partials of rows 12..15.
    weave_ps(p31, [part_filler(10, 0), part_filler(10, 1),
                   part_filler(11, 0), part_filler(11, 1)], keep=3)
    weave_ps(p32, [rest_filler(8, 0), rest_filler(8, 1),
                   rest_filler(9, 0), rest_filler(9, 1)], keep=3)
    weave_pair(3, 3, [rest_filler(10, 0), rest_filler(10, 1),
                      rest_filler(11, 0), rest_filler(11, 1)]
               + [part_filler(sq, oc) for sq in (12, 13, 14, 15)
                  for oc in range(2)], pe_transpose=True, keep=4)
    # tail: finish rows 12..15 (dvt=3 slice only), over all freed psum banks,
    # with the copies alternating DVE / ACT (gpsimd cannot read PSUM); row 15
    # first since its transpose lands first.
    for i, sq in enumerate((15, 14, 13, 12)):
        for oc in range(2):
            k = 2 * i + oc
            emit_op_rest(sq, oc, parts[(sq, oc)],
                         psum_st if k % 3 == 0 else psum_cx,
                         nc.vector if k % 2 == 0 else nc.scalar)

    return pools


def _build_nc():
    import concourse.tile as tile
    from concourse import bacc, mybir

    dt = mybir.dt
    nc = bacc.Bacc("TRN2", target_bir_lowering=False, debug=False,
                   num_devices=NCORES)
    xT = nc.dram_tensor("xt", [D, S], dt.bfloat16, kind="ExternalInput")
    wq = nc.dram_tensor("wq", [D, GD], dt.bfloat16, kind="ExternalInput")
    wk = nc.dram_tensor("wk", [D, GD], dt.bfloat16, kind="ExternalInput")
    wv = nc.dram_tensor("wv", [D, GD], dt.bfloat16, kind="ExternalInput")
    ow = nc.dram_tensor("ow", [GD, D], dt.bfloat16, kind="ExternalInput")
    outp = nc.dram_tensor("outp", [S, D], dt.bfloat16, kind="ExternalOutput")

    with tile.TileContext(nc) as tc:
        pools = _build_body(tc, nc, mybir, xT, wq, wk, wv, ow, outp)
        pools.close()
    nc.compile()
    return nc


LAST_RESULTS = None


def kernel(batch, w_query, w_key, w_value, out_w, out_b):
    global LAST_RESULTS
    import os
    from concourse import bass_utils

    try:  # BASS_TRACE needs the axon NTFF hook; without it the run crashes
        from antenv.axon_hooks import get_axon_ntff_profile_hook  # noqa: F401
    except ImportError:
        os.environ.setdefault("BASS_NEVER_TRACE", "1")

    batch = np.asarray(batch, dtype=np.float32)
    w_query = np.asarray(w_query, dtype=np.float32)
    w_key = np.asarray(w_key, dtype=np.float32)
    w_value = np.asarray(w_value, dtype=np.float32)
    out_w = np.asarray(out_w, dtype=np.float32)
    out_b = np.asarray(out_b, dtype=np.float32)

    if "nc" not in _cache:
        _cache["nc"] = _build_nc()
    nc = _cache["nc"]

    xts = [np.ascontiguousarray(batch[b].T).astype(BF16) for b in range(B)]
    slc = [slice(g * GD, (g + 1) * GD) for g in range(2)]
    wqs = [np.ascontiguousarray(w_query[:, s]).astype(BF16) for s in slc]
    wks = [np.ascontiguousarray(w_key[:, s]).astype(BF16) for s in slc]
    wvs = [np.ascontiguousarray(w_value[:, s]).astype(BF16) for s in slc]
    ows = [np.ascontiguousarray(out_w[s, :]).astype(BF16) for s in slc]
    in_maps = []
    for c in range(NCORES):
        b, g = divmod(c, 2)
        in_maps.append({
            "xt": xts[b], "wq": wqs[g], "wk": wks[g],
            "wv": wvs[g], "ow": ows[g],
        })

    res = bass_utils.run_bass_kernel_spmd(
        nc, in_maps, core_ids=list(range(NCORES)),
    )
    LAST_RESULTS = res

    out = np.empty((B, S, D), np.float32)
    for b in range(B):
        out[b] = res.results[2 * b]["outp"].astype(np.float32) \
            + res.results[2 * b + 1]["outp"].astype(np.float32) \
            + out_b[None, :]
    return out
